# revision 4
# baseline (speedup 1.0000x reference)
"""CoxPH loss (nn_CoxPHLoss) on 8 Trainium2 NeuronCores via Bass.

Contract: kernel(risk, time, event) -> np.float32 scalar, matching

    order = argsort(-time); r = risk[order]; e = event[order] > 0
    clse = cumulative logsumexp of r (descending-time order)
    log_denom_i = clse[last index of i's time-tie group]
    nll = sum_{i: e_i} (log_denom_i - r_i)      (0.0 if no events)

Because time takes integer values in [0, 4096), the tie-group denominator
for time value t is SE_t = sum_{j: time_j >= t} exp(risk_j), so

    nll = sum_t d_t * log(SE_t) - sum_i event_i * risk_i,  d_t = #events at t.

Distribution (data-parallel per the sharding hint): the host sorts by
descending time (16-bit radix argsort) and shards the sorted stream over the
8 cores. Each core runs the memory-bound pass over its 1M-sample shard in a
partition-major layout (sorted element g at [g % 128, g // 128], fp8-e3m4):

  - exp() of every element, split across THREE engines working on disjoint
    column ranges in parallel: ScalarE (exact LUT exp), VectorE and GpSimd
    (Schraudolph exp: one tensor_scalar computing round(y*2^10)+bias into
    int16, whose bits ARE the fp16 value 2^y),
  - per-128-block sums via TensorE: each 128-column group of the exp scratch
    becomes the stationary operand of a ones-vector matmul, so the PE's
    partition-dim reduction yields 128 block sums per instruction,
  - one PSUM->SBUF copy on VectorE and a single fp32 result DMA [128, 64].

The cross-device "carry exchange" is the host-side exclusive cumsum over the
65536 block sums (each block = 128 consecutive sorted samples), after which
each time-group boundary's log-denominator is rebuilt as
base[block] + sum(exp(tail)) with a <=128-element exact host tail. The final
all-reduce is the host dot d_t . log(SE_t) minus sum(risk[event>0]).
"""

import sys

sys.path.insert(0, "/opt/trn_rl_repo")

import numpy as np
import ml_dtypes

import concourse.bacc as bacc
import concourse.mybir as mybir
import concourse.tile as tile
from concourse import bass_utils

P = 128            # SBUF partitions
N_CORES = 8
T_MAX = 4096
F = 8192           # columns per partition-row (per core: P*F = 1M elems)
N = N_CORES * P * F
NG = F // P        # 64 column groups of 128 -> 64 block-sum outputs
N_CHUNKS = 4

# per-chunk column-group split (groups of 128 cols) per engine:
# ACT 25 total, DVE 23, Pool 16  (balances 0.833 / 1.042 / 1.389 ns/elem)
CHUNK_SPLITS = [(7, 6, 4), (6, 6, 4), (6, 6, 4), (6, 5, 4)]

LOG2E = float(np.log2(np.e))
SCH_SCALE = 1024.0 * LOG2E   # Schraudolph multiplier (fp16 bit domain)
SCH_BIAS = 15360.0 - 59.0    # exponent bias + mean-error tuning
CLIP_LO, CLIP_HI = -9.5, 9.5

_cache = {}


def _build_kernel():
    """Per-core SPMD kernel.

    in:  r8 [P, F] fp8-e3m4, partition-major sorted risks
    out: bs [P, NG] fp32 -- bs[m, G] = sum_k exp(r8[k, 128*G + m]), i.e. the
         sum over the 128-sample sorted block with index b = 128*G + m.
    """
    nc = bacc.Bacc("TRN2", target_bir_lowering=False, debug=False)
    r8_d = nc.dram_tensor("r8", [P, F], mybir.dt.float8e3, kind="ExternalInput")
    bs_d = nc.dram_tensor("bs", [P, NG], mybir.dt.float32, kind="ExternalOutput")

    assert sum(sum(s) for s in CHUNK_SPLITS) == NG

    with tile.TileContext(nc) as tc:
        with (
            tc.tile_pool(name="io", bufs=N_CHUNKS) as io,
            tc.tile_pool(name="work", bufs=N_CHUNKS) as work,
            tc.tile_pool(name="acc", bufs=1) as acc,
            tc.tile_pool(name="psum", bufs=1, space="PSUM") as psum,
        ):
            ones_w = acc.tile([P, 1], mybir.dt.float16)
            nc.gpsimd.memset(ones_w[:], 1.0)
            bs_ps = psum.tile([P, NG], mybir.dt.float32)

            gbase = 0
            off = 0
            for c in range(N_CHUNKS):
                na, nd, npl = CHUNK_SPLITS[c]
                FC = sum(CHUNK_SPLITS[c]) * P
                rt = io.tile([P, FC], mybir.dt.float8e3, tag="rt",
                             padded_shape=[P, 17 * P])
                nc.sync.dma_start(rt[:], r8_d[:, off : off + FC])
                off += FC

                ca, cd, cp = na * P, nd * P, npl * P
                ex_a = work.tile([P, ca], mybir.dt.float16, tag="exa",
                                 padded_shape=[P, 7 * P])
                nc.scalar.activation(ex_a[:], rt[:, :ca],
                                     mybir.ActivationFunctionType.Exp)
                ex_d = work.tile([P, cd], mybir.dt.int16, tag="exd",
                                 padded_shape=[P, 6 * P])
                nc.vector.tensor_scalar(ex_d[:], rt[:, ca : ca + cd],
                                        SCH_SCALE, SCH_BIAS,
                                        mybir.AluOpType.mult, mybir.AluOpType.add)
                ex_p = work.tile([P, cp], mybir.dt.int16, tag="exp",
                                 padded_shape=[P, 4 * P])
                nc.gpsimd.tensor_scalar(ex_p[:], rt[:, ca + cd :],
                                        SCH_SCALE, SCH_BIAS,
                                        mybir.AluOpType.mult, mybir.AluOpType.add)

                for src, n in ((ex_a, na), (ex_d, nd), (ex_p, npl)):
                    w = src[:] if src.dtype == mybir.dt.float16 \
                        else src.bitcast(mybir.dt.float16)
                    for g in range(n):
                        nc.tensor.matmul(
                            bs_ps[:, gbase : gbase + 1],
                            w[:, g * P : (g + 1) * P],
                            ones_w[:], start=True, stop=True)
                        gbase += 1

            bs_sb = acc.tile([P, NG], mybir.dt.float32)
            nc.vector.tensor_copy(bs_sb[:], bs_ps[:])
            nc.sync.dma_start(bs_d[:, :], bs_sb[:])

    nc.compile()
    return nc


def _get_kernel():
    if "nc" not in _cache:
        _cache["nc"] = _build_kernel()
    return _cache["nc"]


def kernel(risk: np.ndarray, time: np.ndarray, event: np.ndarray) -> np.float32:
    risk = np.asarray(risk, dtype=np.float32)
    time = np.asarray(time)
    event = np.asarray(event)
    if time.dtype.kind == "u":          # unsigned would wrap under negation
        time = time.astype(np.int64)
    assert risk.shape[0] == N, f"expected N={N}, got {risk.shape}"

    ev = event > 0
    if not bool(ev.any()):
        return np.float32(0.0)

    # host sharding step: descending-time sort (16-bit-key radix argsort)
    order = np.argsort((-time).astype(np.int16), kind="stable")
    rs = risk[order]
    rs_c = np.clip(rs, CLIP_LO, CLIP_HI)
    r8 = rs_c.astype(ml_dtypes.float8_e3m4)

    per_core = P * F
    nc = _get_kernel()
    in_maps = []
    for c in range(N_CORES):
        sh = r8[c * per_core : (c + 1) * per_core]
        # partition-major: element g -> [g % P, g // P]
        in_maps.append({"r8": np.ascontiguousarray(sh.reshape(F, P).T)})

    res = bass_utils.run_bass_kernel_spmd(
        nc, in_maps, core_ids=list(range(N_CORES)))

    # device block sums -> flat per-128-sample block sums, in sorted order
    blocks = np.empty(N // P, dtype=np.float64)
    for c in range(N_CORES):
        bs = np.asarray(res.results[c]["bs"]).astype(np.float64)  # [m, G]
        blocks[c * (per_core // P) : (c + 1) * (per_core // P)] = \
            np.ascontiguousarray(bs.T).ravel()     # b = 128*G + m

    # host combine: exclusive prefix over block sums (the carry exchange)
    base = np.concatenate(([0.0], np.cumsum(blocks)[:-1]))

    # time-group boundaries in the descending-sorted stream
    cnt_desc = np.bincount(time, minlength=T_MAX)[::-1]
    ends = np.cumsum(cnt_desc)
    d_desc = np.bincount(time[ev], minlength=T_MAX)[::-1].astype(np.float64)
    mask = d_desc > 0
    s = ends[mask] - 1                   # last sorted index of each event group

    b = s >> 7                           # block containing the boundary
    # exact host tail: exp of the <=128 in-block elements up to s
    idx = (b[:, None] << 7) + np.arange(P)[None, :]
    tail_mask = idx <= s[:, None]
    tails = (np.exp(rs_c[np.minimum(idx, N - 1)].astype(np.float64))
             * tail_mask).sum(axis=1)

    se = base[b] + tails
    nll = float(np.dot(d_desc[mask], np.log(se))) \
        - float(risk[ev].astype(np.float64).sum())
    return np.float32(nll)


# revision 7
# speedup vs baseline: 1.0102x; 1.0102x over previous
"""CoxPH loss (nn_CoxPHLoss) on 8 Trainium2 NeuronCores via Bass.

Contract: kernel(risk, time, event) -> np.float32 scalar, matching

    order = argsort(-time); r = risk[order]; e = event[order] > 0
    clse = cumulative logsumexp of r (descending-time order)
    log_denom_i = clse[last index of i's time-tie group]
    nll = sum_{i: e_i} (log_denom_i - r_i)      (0.0 if no events)

Because time takes integer values in [0, 4096), the tie-group denominator
for time value t is SE_t = sum_{j: time_j >= t} exp(risk_j), so

    nll = sum_t d_t * log(SE_t) - sum_i event_i * risk_i,  d_t = #events at t.

Distribution (data-parallel per the sharding hint): the host sorts by
descending time (16-bit radix argsort) and shards the sorted stream over the
8 cores. Each core runs the memory-bound pass over its 1M-sample shard in a
partition-major layout (sorted element g at [g % 128, g // 128], fp8-e3m4):

  - exp() of every element, split across THREE engines working on disjoint
    column ranges in parallel: ScalarE (exact LUT exp), VectorE and GpSimd
    (Schraudolph exp: one tensor_scalar computing round(y*2^10)+bias into
    int16, whose bits ARE the fp16 value 2^y),
  - per-128-block sums via TensorE: each 128-column group of the exp scratch
    becomes the stationary operand of a ones-vector matmul, so the PE's
    partition-dim reduction yields 128 block sums per instruction,
  - one PSUM->SBUF copy on VectorE and a single fp32 result DMA [128, 64].

The cross-device "carry exchange" is the host-side exclusive cumsum over the
65536 block sums (each block = 128 consecutive sorted samples), after which
each time-group boundary's log-denominator is rebuilt as
base[block] + sum(exp(tail)) with a <=128-element exact host tail. The final
all-reduce is the host dot d_t . log(SE_t) minus sum(risk[event>0]).
"""

import sys

sys.path.insert(0, "/opt/trn_rl_repo")

import numpy as np
import ml_dtypes

import concourse.bacc as bacc
import concourse.mybir as mybir
import concourse.tile as tile
from concourse import bass_utils

P = 128            # SBUF partitions
N_CORES = 8
T_MAX = 4096
F = 8192           # columns per partition-row (per core: P*F = 1M elems)
N = N_CORES * P * F
NG = F // P        # 64 column groups of 128 -> 64 block-sum outputs
N_CHUNKS = 4

# per-chunk column-group split (groups of 128 cols) per engine (ACT, DVE, Pool):
# ACT 18 total, DVE 34, Pool 12 (rates 0.833 / 0.521 / 1.389 ns/elem; DVE gets
# the 2x_2p DVE mode). Chunks are uneven: small first (earliest compute start)
# and small last (shortest post-stream straggler).
CHUNK_SPLITS = [(2, 5, 1), (5, 11, 4), (7, 11, 4), (4, 7, 3)]
# PSUM evacuation points: copy groups [lo, hi) to SBUF after chunk index k
EVAC_AFTER = {1: (0, 28), 3: (28, 64)}

LOG2E = float(np.log2(np.e))
SCH_SCALE = 1024.0 * LOG2E   # Schraudolph multiplier (fp16 bit domain)
SCH_BIAS = 15360.0 - 59.0    # exponent bias + mean-error tuning
CLIP_LO, CLIP_HI = -9.5, 9.5

_cache = {}


def _build_kernel():
    """Per-core SPMD kernel.

    in:  r8 [P, F] fp8-e3m4, partition-major sorted risks
    out: bs [P, NG] fp32 -- bs[m, G] = sum_k exp(r8[k, 128*G + m]), i.e. the
         sum over the 128-sample sorted block with index b = 128*G + m.
    """
    nc = bacc.Bacc("TRN2", target_bir_lowering=False, debug=False)
    r8_d = nc.dram_tensor("r8", [P, F], mybir.dt.float8e3, kind="ExternalInput")
    bs_d = nc.dram_tensor("bs", [P, NG], mybir.dt.float32, kind="ExternalOutput")

    assert sum(sum(s) for s in CHUNK_SPLITS) == NG

    with tile.TileContext(nc) as tc:
        with (
            tc.tile_pool(name="io", bufs=N_CHUNKS) as io,
            tc.tile_pool(name="work", bufs=N_CHUNKS) as work,
            tc.tile_pool(name="acc", bufs=1) as acc,
            tc.tile_pool(name="psum", bufs=1, space="PSUM") as psum,
        ):
            ones_w = acc.tile([P, 1], mybir.dt.float16)
            nc.gpsimd.memset(ones_w[:], 1.0)
            bs_ps = psum.tile([P, NG], mybir.dt.float32)

            bs_sb = acc.tile([P, NG], mybir.dt.float32)

            max_chunk = max(sum(s) for s in CHUNK_SPLITS)
            gbase = 0
            off = 0
            for c in range(N_CHUNKS):
                na, nd, npl = CHUNK_SPLITS[c]
                FC = sum(CHUNK_SPLITS[c]) * P
                rt = io.tile([P, FC], mybir.dt.float8e3, tag="rt",
                             padded_shape=[P, max_chunk * P])
                nc.sync.dma_start(rt[:], r8_d[:, off : off + FC])
                off += FC

                max_a = max(s[0] for s in CHUNK_SPLITS) * P
                max_d = max(s[1] for s in CHUNK_SPLITS) * P
                max_p = max(s[2] for s in CHUNK_SPLITS) * P
                ca, cd, cp = na * P, nd * P, npl * P
                ex_a = work.tile([P, ca], mybir.dt.float16, tag="exa",
                                 padded_shape=[P, max_a])
                nc.scalar.activation(ex_a[:], rt[:, :ca],
                                     mybir.ActivationFunctionType.Exp)
                ex_d = work.tile([P, cd], mybir.dt.int16, tag="exd",
                                 padded_shape=[P, max_d])
                nc.vector.tensor_scalar(ex_d[:], rt[:, ca : ca + cd],
                                        SCH_SCALE, SCH_BIAS,
                                        mybir.AluOpType.mult, mybir.AluOpType.add)
                ex_p = work.tile([P, cp], mybir.dt.int16, tag="exp",
                                 padded_shape=[P, max_p])
                nc.gpsimd.tensor_scalar(ex_p[:], rt[:, ca + cd :],
                                        SCH_SCALE, SCH_BIAS,
                                        mybir.AluOpType.mult, mybir.AluOpType.add)

                for src, n in ((ex_a, na), (ex_d, nd), (ex_p, npl)):
                    w = src[:] if src.dtype == mybir.dt.float16 \
                        else src.bitcast(mybir.dt.float16)
                    for g in range(n):
                        nc.tensor.matmul(
                            bs_ps[:, gbase : gbase + 1],
                            w[:, g * P : (g + 1) * P],
                            ones_w[:], start=True, stop=True)
                        gbase += 1

                if c in EVAC_AFTER:
                    lo, hi = EVAC_AFTER[c]
                    nc.vector.tensor_copy(bs_sb[:, lo:hi], bs_ps[:, lo:hi])

            nc.sync.dma_start(bs_d[:, :], bs_sb[:])

    nc.compile()
    return nc


def _get_kernel():
    if "nc" not in _cache:
        _cache["nc"] = _build_kernel()
    return _cache["nc"]


def kernel(risk: np.ndarray, time: np.ndarray, event: np.ndarray) -> np.float32:
    risk = np.asarray(risk, dtype=np.float32)
    time = np.asarray(time)
    event = np.asarray(event)
    if time.dtype.kind == "u":          # unsigned would wrap under negation
        time = time.astype(np.int64)
    assert risk.shape[0] == N, f"expected N={N}, got {risk.shape}"

    ev = event > 0
    if not bool(ev.any()):
        return np.float32(0.0)

    # host sharding step: descending-time sort (16-bit-key radix argsort)
    order = np.argsort((-time).astype(np.int16), kind="stable")
    rs = risk[order]
    rs_c = np.clip(rs, CLIP_LO, CLIP_HI)
    r8 = rs_c.astype(ml_dtypes.float8_e3m4)

    per_core = P * F
    nc = _get_kernel()
    in_maps = []
    for c in range(N_CORES):
        sh = r8[c * per_core : (c + 1) * per_core]
        # partition-major: element g -> [g % P, g // P]
        in_maps.append({"r8": np.ascontiguousarray(sh.reshape(F, P).T)})

    res = bass_utils.run_bass_kernel_spmd(
        nc, in_maps, core_ids=list(range(N_CORES)))

    # device block sums -> flat per-128-sample block sums, in sorted order
    blocks = np.empty(N // P, dtype=np.float64)
    for c in range(N_CORES):
        bs = np.asarray(res.results[c]["bs"]).astype(np.float64)  # [m, G]
        blocks[c * (per_core // P) : (c + 1) * (per_core // P)] = \
            np.ascontiguousarray(bs.T).ravel()     # b = 128*G + m

    # host combine: exclusive prefix over block sums (the carry exchange)
    base = np.concatenate(([0.0], np.cumsum(blocks)[:-1]))

    # time-group boundaries in the descending-sorted stream
    cnt_desc = np.bincount(time, minlength=T_MAX)[::-1]
    ends = np.cumsum(cnt_desc)
    d_desc = np.bincount(time[ev], minlength=T_MAX)[::-1].astype(np.float64)
    mask = d_desc > 0
    s = ends[mask] - 1                   # last sorted index of each event group

    b = s >> 7                           # block containing the boundary
    # exact host tail: exp of the <=128 in-block elements up to s
    idx = (b[:, None] << 7) + np.arange(P)[None, :]
    tail_mask = idx <= s[:, None]
    tails = (np.exp(rs_c[np.minimum(idx, N - 1)].astype(np.float64))
             * tail_mask).sum(axis=1)

    se = base[b] + tails
    nll = float(np.dot(d_desc[mask], np.log(se))) \
        - float(risk[ev].astype(np.float64).sum())
    return np.float32(nll)
